# revision 19
# baseline (speedup 1.0000x reference)
"""Trainium2 Bass kernel for nn_Net_19387482374339.

Net: per-batch-element scalar LSTM (IN=1, HID=1) over SEQ=3 steps, then a
Linear(18 -> 1) over flattened groups of 6 consecutive batch elements.

Strategy (v5):
  - Pure data parallel over 8 NeuronCores (batch split).
  - Host converts x to fp16 and rearranges into partition-major layout:
    126 partitions = 21 group-blocks x 6 members; the output linear layer
    becomes 3 TensorE matmuls (contraction over partitions) into PSUM.
  - Classic LSTM gates on ACT (sigmoid/tanh with scale/bias folding); the
    gate pre-activation v is built with a 4x-mode tensor_scalar plus a
    2x-mode tensor_tensor add, choosing v = x*(w/u)+h (scale=u) or
    v = h*(u/w)+x (scale=w) per gate so the folded ratio is <= 1 (fp16 safe).
  - Cell tanh + output-gate product fused into ONE custom DVE op
    (HTMUL5: h = o * (C*(c1 + c3*C^2 + c5*C^4))), with the deg-5 odd
    polynomial fitted at build time to the empirical cell distribution.
  - Pool engine (gpsimd) absorbs some tensor_tensor adds.
  - PSUM drained to fp16 (ACT copy / DVE copy alternating by tile parity).
"""

import numpy as np

N_CORES = 8
B = 12582912
SEQ = 3
Bc = B // N_CORES            # 1,572,864 elements per core
GC = Bc // 6                 # 262,144 output groups per core
NP = 126                     # SBUF partitions used (21 groups of 6)
NQ = 21                      # group blocks
T = 7                        # tiles per core
F = 1786                     # elements per partition per tile
PAD_E = T * NP * F           # 1,575,252 padded elements per core

_CACHE = {}
_OPS = {}


def _register_ops():
    """Register the custom fused DVE ops used by this kernel."""
    if _OPS:
        return _OPS
    import concourse.dve_ops as dve_ops
    from concourse.dve_ops import DveOp
    from concourse.dve_spec import (
        Spec, Src0, Src1, C0, C1, C2, sq, lower, _has_src1,
    )
    from concourse.dve_uop import DveOpSpec

    def register(name, spec, subdim=False):
        if name in dve_ops._SUB_OPCODE_FOR_NAME:
            return next(o for o in dve_ops.OPS if o.name == name)
        row = dve_ops._CUSTOM_DVE_ROW_BASE + len(dve_ops.OPS)
        assert row < 0x20
        shas = {}
        for ver in ("v3", "v4"):
            try:
                tmp = DveOpSpec(name=name, opcode=row, uops=lower(spec, ver=ver),
                                rd1_en=_has_src1(spec))
                shas[ver] = tmp.sha(ver)
            except Exception:
                pass
        op = DveOp(name, spec, subdim, uops_sha=shas)
        dve_ops.OPS.append(op)
        dve_ops._SUB_OPCODE_FOR_NAME[name] = row
        dve_ops.CUSTOM_DVE_SPECS[name] = spec
        return op

    # out = Src1 * (Src0 * (c1 + c3*z2 + c5*z2^2)); z2 = Src0^2
    def _htmul5_body():
        z2 = sq(Src0)
        t = C2 * z2 + C1
        t = t * z2 + C0
        y = t * Src0
        return y * Src1

    _OPS["HTMUL5"] = register(
        "HTMUL5_ANT",
        Spec(body=_htmul5_body(),
             reference=lambda in0, in1, s0, s1, imm2: in1 * (
                 ((imm2 * in0.astype(np.float64)**2 + s1)
                  * in0.astype(np.float64)**2 + s0) * in0)),
    )
    return _OPS


def _fit_cell_polys(wi, wf, wg, wo, ui, uf, ug, uo, bi, bf, bg, bo):
    """Simulate the fp16 pipeline on random normal samples; fit deg-5 odd
    polynomials for tanh over each cell-state distribution."""
    rng = np.random.default_rng(12345)
    N = 400000
    x = rng.standard_normal((N, 3))
    # match the true input tail (B=12.6M standard normals reach ~5.5 sigma)
    x[:12] = 5.6
    x[12:24] = -5.6
    f16 = lambda a: a.astype(np.float16).astype(np.float64)
    sig = lambda z: 1 / (1 + np.exp(-z))
    xh = f16(x)
    params = dict(i=(wi, ui, bi), f=(wf, uf, bf), g=(wg, ug, bg), o=(wo, uo, bo))
    h = np.zeros(N)
    c = np.zeros(N)
    Cs = []
    for t in range(3):
        xt = xh[:, t]
        gates = {}
        for nm, (w_, u_, b_) in params.items():
            if t == 0 or abs(u_) <= 0.02:
                z = w_ * xt + b_
            elif abs(u_) >= abs(w_):
                z = u_ * f16(f16(xt * (w_ / u_)) + h) + b_
            else:
                z = w_ * f16(f16(h * (u_ / w_)) + xt) + b_
            act = np.tanh if nm == "g" else sig
            gates[nm] = f16(act(z))
        if t == 0:
            c = f16(gates["i"] * gates["g"])
        else:
            c = f16(f16(gates["i"] * gates["g"]) + f16(gates["f"] * c))
        Cs.append(c.copy())
        th = f16(np.tanh(c))
        h = f16(gates["o"] * th)

    fits = []
    for c in Cs:
        lim = np.abs(c).max() * 1.03 + 2e-3
        zs = np.concatenate([c[:200000], np.linspace(-lim, lim, 4000)])
        w = np.concatenate([np.ones(min(len(c), 200000)),
                            0.02 * np.ones(4000) * min(len(c), 200000) / 4000])
        A = np.stack([zs ** (2 * k + 1) for k in range(3)], 1)
        sw = np.sqrt(w)[:, None]
        cf, *_ = np.linalg.lstsq(A * sw, np.tanh(zs) * sw[:, 0], rcond=None)
        fits.append(tuple(float(v) for v in cf))
    return fits


def _build_kernel(key):
    (wi, wf, wg, wo, ui, uf, ug, uo, bi, bf, bg, bo) = key
    import concourse.bacc as bacc
    import concourse.tile as tile
    from concourse import mybir

    ops = _register_ops()
    HTMUL5 = ops["HTMUL5"]
    fits = _fit_cell_polys(*key)

    dt = mybir.dt
    AF = mybir.ActivationFunctionType
    ALU = mybir.AluOpType
    F16 = dt.float16

    nc = bacc.Bacc("TRN2", target_bir_lowering=False, debug=False)

    # Register activation-bias constants (bias APs must pre-exist).
    for v in sorted({float(v) for v in (bi, bf, bg, bo)}):
        t = nc.alloc_sbuf_tensor(f"const-user-{v!r}", [128, 1], dt.float32)
        nc.gpsimd.memset(t.ap(), v)
        nc.const_aps.aps[(dt.float32, v)] = t.ap()
    nc.all_engine_barrier()

    xds = [nc.declare_dram_parameter("x0", [T, NP, F], F16, isOutput=False), None, None]
    gnames = ("i", "f", "g", "o")
    _gw = dict(i=(wi, ui), f=(wf, uf), g=(wg, ug), o=(wo, uo))
    DROP_H = {g for g in gnames if abs(_gw[g][1]) <= 0.02}   # negligible recurrence
    gvars = {}   # (gname, sti) -> dram param for host-prescaled x
    need_raw = False
    for sti in (1, 2):
        for gname in gnames:
            w_, u_ = _gw[gname]
            if gname in DROP_H:
                need_raw = True
            elif abs(u_) >= abs(w_):
                gvars[(gname, sti)] = nc.declare_dram_parameter(
                    f"xs{gname}{sti}", [T, NP, F], F16, isOutput=False)
            else:
                need_raw = True
    if need_raw:
        xds[1] = nc.declare_dram_parameter("x1", [T, NP, F], F16, isOutput=False)
        xds[2] = nc.declare_dram_parameter("x2", [T, NP, F], F16, isOutput=False)
    wds = [nc.declare_dram_parameter(f"w{t + 1}", [NP, NQ], F16, isOutput=False)
           for t in range(3)]
    PE_G = "g" not in DROP_H and any(
        abs(_gw["g"][1]) < abs(_gw["g"][0]) for _ in (0,))
    if PE_G:
        wgr_d = nc.declare_dram_parameter("wgr", [NP, NP], F16, isOutput=False)
        wid_d = nc.declare_dram_parameter("wid", [NP, NP], F16, isOutput=False)
    outd = nc.declare_dram_parameter("out", [T, NQ, F], F16, isOutput=True)

    # per-gate config: (name, w, u, b, ACT func)
    gates = (("i", wi, ui, bi, AF.Sigmoid),
             ("f", wf, uf, bf, AF.Sigmoid),
             ("g", wg, ug, bg, AF.Tanh),
             ("o", wo, uo, bo, AF.Sigmoid))

    with tile.TileContext(nc) as tc:
        with tc.tile_pool(name="wpool", bufs=1) as wpool, \
             tc.tile_pool(name="sbuf", bufs=2) as pool, \
             tc.tile_pool(name="psum", bufs=2, space="PSUM") as psum_pool:
            wt = []
            for wd in wds:
                w = wpool.tile([NP, NQ], F16, tag=f"w{wd.name}")
                nc.sync.dma_start(w[:], wd[:])
                wt.append(w)
            if PE_G:
                wgr = wpool.tile([NP, NP], F16, tag="wgr")
                nc.sync.dma_start(wgr[:], wgr_d[:])
                wid = wpool.tile([NP, NP], F16, tag="wid")
                nc.sync.dma_start(wid[:], wid_d[:])

            def lin_matmuls(st, ti):
                pt, hs = st["pt"], st["hs"]
                c0 = 0
                while c0 < F:
                    cw = min(512, F - c0)
                    nc.tensor.matmul(
                        pt[:, c0:c0 + cw],
                        wt[ti][:],
                        hs[ti][:, c0:c0 + cw],
                        start=(ti == 0),
                        stop=(ti == 2),
                    )
                    c0 += cw

            # Pool (gpsimd) handles the v-adds with downstream slack (f, o);
            # i and g feed m1 immediately, keep them on DVE.
            POOL_GATES = ()

            def stage0(k):
                """DMA in, t0 gates on ACT, C1, h0, x-side TS products."""
                st = {"k": k}
                xf = [None, None, None]
                for t in range(3):
                    if xds[t] is None:
                        continue
                    tle = pool.tile([NP, F], F16, tag=f"x{t}", bufs=2 + t, name=f"x{t}_{k}")
                    nc.sync.dma_start(tle[:], xds[t][k])
                    xf[t] = tle
                st["xf"] = xf
                st["xs"] = {}
                for (gname, sti), dram in gvars.items():
                    xs = pool.tile([NP, F], F16, tag=f"xs{gname}{sti}", bufs=3,
                                   name=f"xs{gname}{sti}_{k}")
                    nc.sync.dma_start(xs[:], dram[k])
                    st["xs"][(gname, sti)] = xs
                g0 = {}
                for gname, w, u, b, func in gates:
                    if gname == "f":
                        continue
                    gt = pool.tile([NP, F], F16, tag=f"g{gname}0", bufs=2, name=f"{gname}0_{k}")
                    nc.scalar.activation(gt[:], xf[0][:], func, bias=float(b), scale=float(w))
                    g0[gname] = gt
                c1 = pool.tile([NP, F], F16, tag="c", bufs=5, name=f"c1_{k}")
                nc.vector.tensor_tensor(c1[:], g0["i"][:], g0["g"][:], ALU.mult)
                h0 = pool.tile([NP, F], F16, tag="h0", bufs=4, name=f"h0_{k}")
                cf = fits[0]
                nc.vector._custom_dve(HTMUL5, out=h0[:], in0=c1[:], in1=g0["o"][:],
                                      s0=cf[0], s1=cf[1], imm2=cf[2])
                st["h"] = h0
                st["c"] = c1
                st["hs"] = [h0]
                return st

            def step(st, sti):
                """One LSTM timestep (sti = 1 or 2)."""
                k = st["k"]
                xt = st["xf"][sti]
                hprev = st["hs"][sti - 1]
                cprev = st["c"]
                vs = {}
                vg_psum = None
                for gname, w, u, b, func in gates:
                    if gname in DROP_H:
                        continue
                    xs = st["xs"].get((gname, sti))
                    if xs is not None:
                        vs[gname] = (xs, hprev)
                    elif PE_G and gname == "g":
                        vg = psum_pool.tile([NP, F], dt.float32, tag="vg",
                                            bufs=1, name=f"vg{sti}_{k}")
                        c0 = 0
                        while c0 < F:
                            c1 = min(c0 + 512, F)
                            nc.tensor.matmul(vg[:, c0:c1], wgr[:], hprev[:, c0:c1],
                                             start=True, stop=False)
                            nc.tensor.matmul(vg[:, c0:c1], wid[:], xt[:, c0:c1],
                                             start=False, stop=True)
                            c0 = c1
                        vs[gname] = ("psum", [(vg, 0, F)])
                    else:
                        ts = pool.tile([NP, F], F16, tag=f"ts{gname}{sti}", bufs=2,
                                       name=f"ts{gname}{sti}_{k}")
                        nc.vector.tensor_scalar(ts[:], hprev[:], float(u / w), None, ALU.mult)
                        vs[gname] = (ts, xt)
                gv = {}
                # emit critical gates (i, g) first so ACT starts early
                order = sorted(gates, key=lambda t: 0 if t[0] in ("i", "g") else 1)
                for gname, w, u, b, func in order:
                    gt = pool.tile([NP, F], F16, tag=f"g{gname}", bufs=2, name=f"{gname}{sti}_{k}")
                    if gname in DROP_H:
                        nc.scalar.activation(gt[:], xt[:], func, bias=float(b), scale=float(w))
                        gv[gname] = gt
                        continue
                    ts, other = vs[gname]
                    if ts == "psum":
                        for vg, c0, c1 in other:
                            nc.scalar.activation(gt[:, c0:c1], vg[:, :c1 - c0], func,
                                                 bias=float(b), scale=float(w))
                        gv[gname] = gt
                        continue
                    eng = nc.gpsimd if gname in POOL_GATES else nc.vector
                    eng.tensor_tensor(ts[:], ts[:], other[:], ALU.add)
                    scale = u if abs(u) >= abs(w) else w
                    nc.scalar.activation(gt[:], ts[:], func, bias=float(b), scale=float(scale))
                    gv[gname] = gt
                m1 = gv["i"]
                nc.vector.tensor_tensor(m1[:], gv["i"][:], gv["g"][:], ALU.mult)
                m2 = gv["f"]
                nc.vector.tensor_tensor(m2[:], gv["f"][:], cprev[:], ALU.mult)
                cn = pool.tile([NP, F], F16, tag="c", bufs=5, name=f"c{sti + 1}_{k}")
                nc.vector.tensor_tensor(cn[:], m1[:], m2[:], ALU.add)
                hn = pool.tile([NP, F], F16, tag=f"h{sti}", bufs=3, name=f"h{sti}_{k}")
                cf = fits[sti]
                nc.vector._custom_dve(HTMUL5, out=hn[:], in0=cn[:], in1=gv["o"][:],
                                      s0=cf[0], s1=cf[1], imm2=cf[2])
                st["c"] = cn
                st["hs"].append(hn)

            def stageA(st):
                step(st, 1)

            pt_shared = {}

            def stageB(st):
                k = st["k"]
                step(st, 2)
                if k % 2 == 0:
                    pt = psum_pool.tile([32 + NQ, F], dt.float32, tag="lin", bufs=1, name=f"pt_{k}")
                    pt_shared["pt"] = pt
                else:
                    pt = pt_shared["pt"]
                base = (k % 2) * 32
                st["pt"] = pt[base:base + NQ, :]
                for ti in range(3):
                    lin_matmuls(st, ti)
                if k % 2 == 1:
                    outs = pool.tile([32 + NQ, F], F16, tag="outs", bufs=2, name=f"outs_{k}")
                    nc.vector.tensor_scalar(outs[:], pt[:], 0.0, None, ALU.add)
                    nc.sync.dma_start(outd[k - 1], outs[:NQ, :])
                    nc.sync.dma_start(outd[k], outs[32:32 + NQ, :])
                elif k == T - 1:
                    outs = pool.tile([32 + NQ, F], F16, tag="outs", bufs=2, name=f"outs_{k}")
                    nc.vector.tensor_scalar(outs[:NQ, :], pt[:NQ, :], 0.0, None, ALU.add)
                    nc.sync.dma_start(outd[k], outs[:NQ, :])

            sts = {}
            for k in range(T + 2):
                if k < T:
                    sts[k] = stage0(k)
                if k - 1 >= 0 and k - 1 < T:
                    stageA(sts[k - 1])
                if k - 2 >= 0:
                    stageB(sts[k - 2])
                    del sts[k - 2]

    nc.finalize()
    return nc


def kernel(x, w_ih, w_hh, b_ih, b_hh, w_lin, b_lin):
    from concourse.bass_utils import run_bass_kernel_spmd

    x = np.asarray(x, dtype=np.float32)
    w_ih = np.asarray(w_ih, dtype=np.float32)
    w_hh = np.asarray(w_hh, dtype=np.float32)
    b_ih = np.asarray(b_ih, dtype=np.float32)
    b_hh = np.asarray(b_hh, dtype=np.float32)
    w_lin = np.asarray(w_lin, dtype=np.float32)
    b_lin = np.asarray(b_lin, dtype=np.float32)

    wi, wf, wg, wo = (float(v) for v in w_ih[:, 0])
    ui, uf, ug, uo = (float(v) for v in w_hh[:, 0])
    bias = b_ih + b_hh
    bi, bf, bg, bo = (float(v) for v in bias)
    wl = w_lin[0]            # [18]
    bl = float(b_lin[0])

    key = (wi, wf, wg, wo, ui, uf, ug, uo, bi, bf, bg, bo)
    if key not in _CACHE:
        _CACHE[key] = _build_kernel(key)
    nc = _CACHE[key]

    # Linear-stage stationaries: W_t[p, q] = wl[3*(p%6) + t] if q == p//6.
    p = np.arange(NP)
    wmats = []
    for t in range(3):
        W = np.zeros((NP, NQ), dtype=np.float16)
        W[p, p // 6] = wl[3 * (p % 6) + t].astype(np.float16)
        wmats.append(W)

    # Host data prep: [B, 3, 1] -> per-core padded [3, T, NP, F] fp16.
    xb = x.reshape(B, SEQ)
    in_maps = []
    for c in range(N_CORES):
        xc = xb[c * Bc:(c + 1) * Bc]
        if PAD_E != Bc:
            xp = np.zeros((PAD_E, SEQ), dtype=np.float32)
            xp[:Bc] = xc
        else:
            xp = xc
        # element e = ((tile*21 + q)*F + j)*6 + b  ->  [tile][q][j][b][t]
        xr32 = xp.reshape(T, NQ, F, 6, SEQ).transpose(4, 0, 1, 3, 2)
        xr = np.ascontiguousarray(xr32, dtype=np.float16).reshape(SEQ, T, NP, F)
        im = {"x0": xr[0], "w1": wmats[0], "w2": wmats[1], "w3": wmats[2]}
        gw = dict(i=(wi, ui), f=(wf, uf), g=(wg, ug), o=(wo, uo))
        need_raw = False
        for sti in (1, 2):
            for gname, (w_, u_) in gw.items():
                if abs(u_) <= 0.02 or abs(u_) < abs(w_):
                    need_raw = True
                else:
                    im[f"xs{gname}{sti}"] = np.ascontiguousarray(
                        xr32[sti] * np.float32(w_ / u_), dtype=np.float16
                    ).reshape(T, NP, F)
        if need_raw:
            im["x1"] = xr[1]
            im["x2"] = xr[2]
        rg = gw["g"][1] / gw["g"][0]
        im["wgr"] = np.ascontiguousarray(np.diag(np.full(NP, rg, np.float32)).astype(np.float16))
        im["wid"] = np.ascontiguousarray(np.eye(NP, dtype=np.float16))
        in_maps.append(im)

    res = run_bass_kernel_spmd(nc, in_maps, list(range(N_CORES)))

    out = np.empty((B // 6, 1), dtype=np.float32)
    for c in range(N_CORES):
        oc = res.results[c]["out"].astype(np.float32).reshape(-1)[:GC]
        out[c * GC:(c + 1) * GC, 0] = oc + bl
    return out


# revision 20
# speedup vs baseline: 1.0446x; 1.0446x over previous
"""Trainium2 Bass kernel for nn_Net_19387482374339.

Net: per-batch-element scalar LSTM (IN=1, HID=1) over SEQ=3 steps, then a
Linear(18 -> 1) over flattened groups of 6 consecutive batch elements.

Strategy (v5):
  - Pure data parallel over 8 NeuronCores (batch split).
  - Host converts x to fp16 and rearranges into partition-major layout:
    126 partitions = 21 group-blocks x 6 members; the output linear layer
    becomes 3 TensorE matmuls (contraction over partitions) into PSUM.
  - Classic LSTM gates on ACT (sigmoid/tanh with scale/bias folding); the
    gate pre-activation v is built with a 4x-mode tensor_scalar plus a
    2x-mode tensor_tensor add, choosing v = x*(w/u)+h (scale=u) or
    v = h*(u/w)+x (scale=w) per gate so the folded ratio is <= 1 (fp16 safe).
  - Cell tanh + output-gate product fused into ONE custom DVE op
    (HTMUL5: h = o * (C*(c1 + c3*C^2 + c5*C^4))), with the deg-5 odd
    polynomial fitted at build time to the empirical cell distribution.
  - Pool engine (gpsimd) absorbs some tensor_tensor adds.
  - PSUM drained to fp16 (ACT copy / DVE copy alternating by tile parity).
"""

import numpy as np

N_CORES = 8
B = 12582912
SEQ = 3
Bc = B // N_CORES            # 1,572,864 elements per core
GC = Bc // 6                 # 262,144 output groups per core
NP = 126                     # SBUF partitions used (21 groups of 6)
NQ = 21                      # group blocks
T = 7                        # tiles per core
F = 1786                     # elements per partition per tile
PAD_E = T * NP * F           # 1,575,252 padded elements per core

_CACHE = {}
_OPS = {}


def _register_ops():
    """Register the custom fused DVE ops used by this kernel."""
    if _OPS:
        return _OPS
    import concourse.dve_ops as dve_ops
    from concourse.dve_ops import DveOp
    from concourse.dve_spec import (
        Spec, Src0, Src1, C0, C1, C2, sq, lower, _has_src1,
    )
    from concourse.dve_uop import DveOpSpec

    def register(name, spec, subdim=False):
        if name in dve_ops._SUB_OPCODE_FOR_NAME:
            return next(o for o in dve_ops.OPS if o.name == name)
        row = dve_ops._CUSTOM_DVE_ROW_BASE + len(dve_ops.OPS)
        assert row < 0x20
        shas = {}
        for ver in ("v3", "v4"):
            try:
                tmp = DveOpSpec(name=name, opcode=row, uops=lower(spec, ver=ver),
                                rd1_en=_has_src1(spec))
                shas[ver] = tmp.sha(ver)
            except Exception:
                pass
        op = DveOp(name, spec, subdim, uops_sha=shas)
        dve_ops.OPS.append(op)
        dve_ops._SUB_OPCODE_FOR_NAME[name] = row
        dve_ops.CUSTOM_DVE_SPECS[name] = spec
        return op

    # out = Src1 * (Src0 * (c1 + c3*z2 + c5*z2^2)); z2 = Src0^2
    def _htmul5_body():
        z2 = sq(Src0)
        t = C2 * z2 + C1
        t = t * z2 + C0
        y = t * Src0
        return y * Src1

    _OPS["HTMUL5"] = register(
        "HTMUL5_ANT",
        Spec(body=_htmul5_body(),
             reference=lambda in0, in1, s0, s1, imm2: in1 * (
                 ((imm2 * in0.astype(np.float64)**2 + s1)
                  * in0.astype(np.float64)**2 + s0) * in0)),
    )
    return _OPS


def _fit_cell_polys(wi, wf, wg, wo, ui, uf, ug, uo, bi, bf, bg, bo):
    """Simulate the fp16 pipeline on random normal samples; fit deg-5 odd
    polynomials for tanh over each cell-state distribution."""
    rng = np.random.default_rng(12345)
    N = 400000
    x = rng.standard_normal((N, 3))
    # match the true input tail (B=12.6M standard normals reach ~5.5 sigma)
    x[:12] = 5.6
    x[12:24] = -5.6
    f16 = lambda a: a.astype(np.float16).astype(np.float64)
    sig = lambda z: 1 / (1 + np.exp(-z))
    xh = f16(x)
    params = dict(i=(wi, ui, bi), f=(wf, uf, bf), g=(wg, ug, bg), o=(wo, uo, bo))
    h = np.zeros(N)
    c = np.zeros(N)
    Cs = []
    for t in range(3):
        xt = xh[:, t]
        gates = {}
        for nm, (w_, u_, b_) in params.items():
            if t == 0 or abs(u_) <= 0.02:
                z = w_ * xt + b_
            elif abs(u_) >= abs(w_):
                z = u_ * f16(f16(xt * (w_ / u_)) + h) + b_
            else:
                z = w_ * f16(f16(h * (u_ / w_)) + xt) + b_
            act = np.tanh if nm == "g" else sig
            gates[nm] = f16(act(z))
        if t == 0:
            c = f16(gates["i"] * gates["g"])
        else:
            c = f16(f16(gates["i"] * gates["g"]) + f16(gates["f"] * c))
        Cs.append(c.copy())
        th = f16(np.tanh(c))
        h = f16(gates["o"] * th)

    fits = []
    for c in Cs:
        lim = np.abs(c).max() * 1.03 + 2e-3
        zs = np.concatenate([c[:200000], np.linspace(-lim, lim, 4000)])
        w = np.concatenate([np.ones(min(len(c), 200000)),
                            0.02 * np.ones(4000) * min(len(c), 200000) / 4000])
        A = np.stack([zs ** (2 * k + 1) for k in range(3)], 1)
        sw = np.sqrt(w)[:, None]
        cf, *_ = np.linalg.lstsq(A * sw, np.tanh(zs) * sw[:, 0], rcond=None)
        fits.append(tuple(float(v) for v in cf))
    return fits


def _build_kernel(key):
    (wi, wf, wg, wo, ui, uf, ug, uo, bi, bf, bg, bo) = key
    import concourse.bacc as bacc
    import concourse.tile as tile
    from concourse import mybir

    ops = _register_ops()
    HTMUL5 = ops["HTMUL5"]
    fits = _fit_cell_polys(*key)

    dt = mybir.dt
    AF = mybir.ActivationFunctionType
    ALU = mybir.AluOpType
    F16 = dt.float16

    nc = bacc.Bacc("TRN2", target_bir_lowering=False, debug=False)

    # Register activation-bias constants (bias APs must pre-exist).
    for v in sorted({float(v) for v in (bi, bf, bg, bo)}):
        t = nc.alloc_sbuf_tensor(f"const-user-{v!r}", [128, 1], dt.float32)
        nc.gpsimd.memset(t.ap(), v)
        nc.const_aps.aps[(dt.float32, v)] = t.ap()
    nc.all_engine_barrier()

    xds = [nc.declare_dram_parameter("x0", [T, NP, F], F16, isOutput=False), None, None]
    gnames = ("i", "f", "g", "o")
    _gw = dict(i=(wi, ui), f=(wf, uf), g=(wg, ug), o=(wo, uo))
    DROP_H = {g for g in gnames if abs(_gw[g][1]) <= 0.02}   # negligible recurrence
    gvars = {}   # (gname, sti) -> dram param for host-prescaled x
    need_raw = False
    for sti in (1, 2):
        for gname in gnames:
            w_, u_ = _gw[gname]
            if gname in DROP_H:
                need_raw = True
            elif abs(u_) >= abs(w_):
                gvars[(gname, sti)] = nc.declare_dram_parameter(
                    f"xs{gname}{sti}", [T, NP, F], F16, isOutput=False)
            else:
                need_raw = True
    if need_raw:
        xds[1] = nc.declare_dram_parameter("x1", [T, NP, F], F16, isOutput=False)
        xds[2] = nc.declare_dram_parameter("x2", [T, NP, F], F16, isOutput=False)
    wds = [nc.declare_dram_parameter(f"w{t + 1}", [NP, NQ], F16, isOutput=False)
           for t in range(3)]
    PE_G = "g" not in DROP_H and any(
        abs(_gw["g"][1]) < abs(_gw["g"][0]) for _ in (0,))
    if PE_G:
        wgr_d = nc.declare_dram_parameter("wgr", [NP, NP], F16, isOutput=False)
        wid_d = nc.declare_dram_parameter("wid", [NP, NP], F16, isOutput=False)
    outd = nc.declare_dram_parameter("out", [T, NQ, F], F16, isOutput=True)

    # per-gate config: (name, w, u, b, ACT func)
    gates = (("i", wi, ui, bi, AF.Sigmoid),
             ("f", wf, uf, bf, AF.Sigmoid),
             ("g", wg, ug, bg, AF.Tanh),
             ("o", wo, uo, bo, AF.Sigmoid))

    with tile.TileContext(nc) as tc:
        with tc.tile_pool(name="wpool", bufs=1) as wpool, \
             tc.tile_pool(name="sbuf", bufs=2) as pool, \
             tc.tile_pool(name="psum", bufs=2, space="PSUM") as psum_pool:
            wt = []
            for wd in wds:
                w = wpool.tile([NP, NQ], F16, tag=f"w{wd.name}")
                nc.sync.dma_start(w[:], wd[:])
                wt.append(w)
            if PE_G:
                wgr = wpool.tile([NP, NP], F16, tag="wgr")
                nc.sync.dma_start(wgr[:], wgr_d[:])
                wid = wpool.tile([NP, NP], F16, tag="wid")
                nc.sync.dma_start(wid[:], wid_d[:])

            def lin_matmuls(st, ti):
                pt, hs = st["pt"], st["hs"]
                c0 = 0
                while c0 < F:
                    cw = min(512, F - c0)
                    nc.tensor.matmul(
                        pt[:, c0:c0 + cw],
                        wt[ti][:],
                        hs[ti][:, c0:c0 + cw],
                        start=(ti == 0),
                        stop=(ti == 2),
                    )
                    c0 += cw

            # Pool (gpsimd) handles the v-adds with downstream slack (f, o);
            # i and g feed m1 immediately, keep them on DVE.
            POOL_GATES = ()

            def stage0(k):
                """DMA in, t0 gates on ACT, C1, h0, x-side TS products."""
                st = {"k": k}
                xf = [None, None, None]
                for t in range(3):
                    if xds[t] is None:
                        continue
                    tle = pool.tile([NP, F], F16, tag=f"x{t}", bufs=2 + t, name=f"x{t}_{k}")
                    nc.sync.dma_start(tle[:], xds[t][k])
                    xf[t] = tle
                st["xf"] = xf
                st["xs"] = {}
                for (gname, sti), dram in gvars.items():
                    xs = pool.tile([NP, F], F16, tag=f"xs{gname}{sti}", bufs=3,
                                   name=f"xs{gname}{sti}_{k}")
                    nc.sync.dma_start(xs[:], dram[k])
                    st["xs"][(gname, sti)] = xs
                g0 = {}
                for gname, w, u, b, func in gates:
                    if gname == "f":
                        continue
                    gt = pool.tile([NP, F], F16, tag=f"g{gname}0", bufs=2, name=f"{gname}0_{k}")
                    nc.scalar.activation(gt[:], xf[0][:], func, bias=float(b), scale=float(w))
                    g0[gname] = gt
                c1 = pool.tile([NP, F], F16, tag="c", bufs=5, name=f"c1_{k}")
                nc.vector.tensor_tensor(c1[:], g0["i"][:], g0["g"][:], ALU.mult)
                h0 = pool.tile([NP, F], F16, tag="h0", bufs=4, name=f"h0_{k}")
                cf = fits[0]
                nc.vector._custom_dve(HTMUL5, out=h0[:], in0=c1[:], in1=g0["o"][:],
                                      s0=cf[0], s1=cf[1], imm2=cf[2])
                st["h"] = h0
                st["c"] = c1
                st["hs"] = [h0]
                return st

            def step(st, sti):
                """One LSTM timestep (sti = 1 or 2)."""
                k = st["k"]
                xt = st["xf"][sti]
                hprev = st["hs"][sti - 1]
                cprev = st["c"]
                vs = {}
                vg_psum = None
                for gname, w, u, b, func in gates:
                    if gname in DROP_H:
                        continue
                    xs = st["xs"].get((gname, sti))
                    if xs is not None:
                        vs[gname] = (xs, hprev)
                    elif PE_G and gname == "g":
                        vg = psum_pool.tile([NP, F], dt.float32, tag="vg",
                                            bufs=1, name=f"vg{sti}_{k}")
                        chunks = []
                        c0 = 0
                        while c0 < F:
                            chunks.append((c0, min(c0 + 512, F)))
                            c0 = min(c0 + 512, F)
                        for c0, c1 in chunks:
                            nc.tensor.matmul(vg[:, c0:c1], wgr[:], hprev[:, c0:c1],
                                             start=True, stop=False)
                        for c0, c1 in chunks:
                            nc.tensor.matmul(vg[:, c0:c1], wid[:], xt[:, c0:c1],
                                             start=False, stop=True)
                        vs[gname] = ("psum", [(vg, 0, F)])
                    else:
                        ts = pool.tile([NP, F], F16, tag=f"ts{gname}{sti}", bufs=2,
                                       name=f"ts{gname}{sti}_{k}")
                        nc.vector.tensor_scalar(ts[:], hprev[:], float(u / w), None, ALU.mult)
                        vs[gname] = (ts, xt)
                gv = {}
                # emit critical gates (i, g) first so ACT starts early
                order = sorted(gates, key=lambda t: 0 if t[0] in ("i", "g") else 1)
                for gname, w, u, b, func in order:
                    gt = pool.tile([NP, F], F16, tag=f"g{gname}", bufs=2, name=f"{gname}{sti}_{k}")
                    if gname in DROP_H:
                        nc.scalar.activation(gt[:], xt[:], func, bias=float(b), scale=float(w))
                        gv[gname] = gt
                        continue
                    ts, other = vs[gname]
                    if ts == "psum":
                        for vg, c0, c1 in other:
                            nc.scalar.activation(gt[:, c0:c1], vg[:, :c1 - c0], func,
                                                 bias=float(b), scale=float(w))
                        gv[gname] = gt
                        continue
                    eng = nc.gpsimd if gname in POOL_GATES else nc.vector
                    eng.tensor_tensor(ts[:], ts[:], other[:], ALU.add)
                    scale = u if abs(u) >= abs(w) else w
                    nc.scalar.activation(gt[:], ts[:], func, bias=float(b), scale=float(scale))
                    gv[gname] = gt
                m1 = gv["i"]
                nc.vector.tensor_tensor(m1[:], gv["i"][:], gv["g"][:], ALU.mult)
                m2 = gv["f"]
                nc.vector.tensor_tensor(m2[:], gv["f"][:], cprev[:], ALU.mult)
                cn = pool.tile([NP, F], F16, tag="c", bufs=5, name=f"c{sti + 1}_{k}")
                nc.vector.tensor_tensor(cn[:], m1[:], m2[:], ALU.add)
                hn = pool.tile([NP, F], F16, tag=f"h{sti}", bufs=3, name=f"h{sti}_{k}")
                cf = fits[sti]
                nc.vector._custom_dve(HTMUL5, out=hn[:], in0=cn[:], in1=gv["o"][:],
                                      s0=cf[0], s1=cf[1], imm2=cf[2])
                st["c"] = cn
                st["hs"].append(hn)

            def stageA(st):
                step(st, 1)

            pt_shared = {}

            def stageB(st):
                k = st["k"]
                step(st, 2)
                if k % 2 == 0:
                    pt = psum_pool.tile([32 + NQ, F], dt.float32, tag="lin", bufs=1, name=f"pt_{k}")
                    pt_shared["pt"] = pt
                else:
                    pt = pt_shared["pt"]
                base = (k % 2) * 32
                st["pt"] = pt[base:base + NQ, :]
                for ti in range(3):
                    lin_matmuls(st, ti)
                if k % 2 == 1:
                    outs = pool.tile([32 + NQ, F], F16, tag="outs", bufs=2, name=f"outs_{k}")
                    nc.vector.tensor_scalar(outs[:], pt[:], 0.0, None, ALU.add)
                    nc.sync.dma_start(outd[k - 1], outs[:NQ, :])
                    nc.sync.dma_start(outd[k], outs[32:32 + NQ, :])
                elif k == T - 1:
                    outs = pool.tile([32 + NQ, F], F16, tag="outs", bufs=2, name=f"outs_{k}")
                    nc.vector.tensor_scalar(outs[:NQ, :], pt[:NQ, :], 0.0, None, ALU.add)
                    nc.sync.dma_start(outd[k][:10], outs[:10, :])
                    nc.sync.dma_start(outd[k][10:], outs[10:NQ, :])

            sts = {}
            for k in range(T + 2):
                if k < T:
                    sts[k] = stage0(k)
                if k - 1 >= 0 and k - 1 < T:
                    stageA(sts[k - 1])
                if k - 2 >= 0:
                    stageB(sts[k - 2])
                    del sts[k - 2]

    nc.finalize()
    return nc


def kernel(x, w_ih, w_hh, b_ih, b_hh, w_lin, b_lin):
    from concourse.bass_utils import run_bass_kernel_spmd

    x = np.asarray(x, dtype=np.float32)
    w_ih = np.asarray(w_ih, dtype=np.float32)
    w_hh = np.asarray(w_hh, dtype=np.float32)
    b_ih = np.asarray(b_ih, dtype=np.float32)
    b_hh = np.asarray(b_hh, dtype=np.float32)
    w_lin = np.asarray(w_lin, dtype=np.float32)
    b_lin = np.asarray(b_lin, dtype=np.float32)

    wi, wf, wg, wo = (float(v) for v in w_ih[:, 0])
    ui, uf, ug, uo = (float(v) for v in w_hh[:, 0])
    bias = b_ih + b_hh
    bi, bf, bg, bo = (float(v) for v in bias)
    wl = w_lin[0]            # [18]
    bl = float(b_lin[0])

    key = (wi, wf, wg, wo, ui, uf, ug, uo, bi, bf, bg, bo)
    if key not in _CACHE:
        _CACHE[key] = _build_kernel(key)
    nc = _CACHE[key]

    # Linear-stage stationaries: W_t[p, q] = wl[3*(p%6) + t] if q == p//6.
    p = np.arange(NP)
    wmats = []
    for t in range(3):
        W = np.zeros((NP, NQ), dtype=np.float16)
        W[p, p // 6] = wl[3 * (p % 6) + t].astype(np.float16)
        wmats.append(W)

    # Host data prep: [B, 3, 1] -> per-core padded [3, T, NP, F] fp16.
    xb = x.reshape(B, SEQ)
    in_maps = []
    for c in range(N_CORES):
        xc = xb[c * Bc:(c + 1) * Bc]
        if PAD_E != Bc:
            xp = np.zeros((PAD_E, SEQ), dtype=np.float32)
            xp[:Bc] = xc
        else:
            xp = xc
        # element e = ((tile*21 + q)*F + j)*6 + b  ->  [tile][q][j][b][t]
        xr32 = xp.reshape(T, NQ, F, 6, SEQ).transpose(4, 0, 1, 3, 2)
        xr = np.ascontiguousarray(xr32, dtype=np.float16).reshape(SEQ, T, NP, F)
        im = {"x0": xr[0], "w1": wmats[0], "w2": wmats[1], "w3": wmats[2]}
        gw = dict(i=(wi, ui), f=(wf, uf), g=(wg, ug), o=(wo, uo))
        need_raw = False
        for sti in (1, 2):
            for gname, (w_, u_) in gw.items():
                if abs(u_) <= 0.02 or abs(u_) < abs(w_):
                    need_raw = True
                else:
                    im[f"xs{gname}{sti}"] = np.ascontiguousarray(
                        xr32[sti] * np.float32(w_ / u_), dtype=np.float16
                    ).reshape(T, NP, F)
        if need_raw:
            im["x1"] = xr[1]
            im["x2"] = xr[2]
        rg = gw["g"][1] / gw["g"][0]
        im["wgr"] = np.ascontiguousarray(np.diag(np.full(NP, rg, np.float32)).astype(np.float16))
        im["wid"] = np.ascontiguousarray(np.eye(NP, dtype=np.float16))
        in_maps.append(im)

    res = run_bass_kernel_spmd(nc, in_maps, list(range(N_CORES)))

    out = np.empty((B // 6, 1), dtype=np.float32)
    for c in range(N_CORES):
        oc = res.results[c]["out"].astype(np.float32).reshape(-1)[:GC]
        out[c * GC:(c + 1) * GC, 0] = oc + bl
    return out
